# revision 1
# baseline (speedup 1.0000x reference)
"""CQAttention Trainium2 kernel — data-parallel over batch across 8 NeuronCores.

Problem shapes (hardcoded): B=32, H=256, Lc=1024, Lq=256.
Each core processes B/8 = 4 batches.

Math (per batch, with all-ones masks — guaranteed by the problem spec):
  Ct = C^T [Lc,H], Qt = Q^T [Lq,H]
  S[l,m] = Ct[l]@w1 + Qt[m]@w2 + (Ct[l]*w3)@Qt[m]
  Z = exp(S) serves BOTH softmaxes:
    S_row = Z / rowsum(Z), S_col = Z / colsum(Z)
  A  = S_row @ Qt
  Bv = S_row @ (S_col^T @ Ct)      (factored: avoids the Lc x Lc product)
  out = relu([Ct, A, Ct*A, Ct*Bv] @ W_res^T + b_res)^T  -> [H, Lc]

v2 structure (vs v1):
  - E_ml is a PE transpose of E_lm (bf16), not a second matmul+exp; its
    PSUM-out copies carry accum_out -> colsums kappa for free. This kills
    the Z^T matmuls, the QA operand, and the q-bias DRAM bounce.
  - The r bias row is computed straight into column layout ([l%128, l/128])
    by 16 tiny matmuls (lhsT=C chunk, rhs=w1), killing the r DRAM bounce.
  - 1/rho is folded into the A^T/Bv^T PSUM->SBUF moves (tensor_tensor with
    a broadcast rinv row) instead of materializing P_ml = E_ml * rinv.
  - Transposes write into grouped [128,512] PSUM tiles -> 1 copy per 4
    transposes instead of 4.
  - Elementwise work is explicitly spread over DVE / Act / Pool(gpsimd):
    Pool takes Q_bf, CA, CB1, kappa adds; Act takes exps, E_ml copies,
    relu, r_col; DVE takes the rest.
  - fp32r for all fp32 matmuls (full PE rate), bf16 for everything
    downstream of exp.
"""

import numpy as np

_CACHE = {}

B_FULL = 32
N_CORES = 8
BB = B_FULL // N_CORES  # batches per core = 4
H = 256
LC = 1024
LQ = 256


def _build(reps: int = 1):
    from contextlib import ExitStack

    import concourse.bass as bass
    import concourse.tile as tile
    from concourse import bacc, mybir
    from concourse.masks import make_identity

    f32 = mybir.dt.float32
    f32r = mybir.dt.float32r
    bf16 = mybir.dt.bfloat16
    AF = mybir.ActivationFunctionType
    OP = mybir.AluOpType

    nc = bacc.Bacc("TRN2", target_bir_lowering=False, debug=False)

    def mm(out, lhsT, rhs, start, stop):
        # fp32r runs the PE at full rate (1 cycle/row for N>=256) vs 4x for fp32
        nc.tensor.matmul(
            out,
            lhsT=lhsT.bitcast(f32r),
            rhs=rhs.bitcast(f32r),
            start=start,
            stop=stop,
        )

    def mmb(out, lhsT, rhs, start, stop):
        nc.tensor.matmul(out, lhsT=lhsT, rhs=rhs, start=start, stop=stop)

    C = nc.dram_tensor("C", [BB, H, LC], f32, kind="ExternalInput")
    Q = nc.dram_tensor("Q", [BB, H, LQ], f32, kind="ExternalInput")
    w = nc.dram_tensor("w", [3 * H], f32, kind="ExternalInput")
    W_res = nc.dram_tensor("W_res", [H, 4 * H], f32, kind="ExternalInput")
    b_res = nc.dram_tensor("b_res", [H], f32, kind="ExternalInput")
    out = nc.dram_tensor("out", [BB, H, LC], f32, kind="ExternalOutput")

    KH = H // 128  # 2 h-chunks
    NLT = LC // 128  # 8 l-tiles
    NMT = LQ // 128  # 2 m-tiles

    with tile.TileContext(nc) as tc:
        with ExitStack() as ctx:
            singles = ctx.enter_context(tc.tile_pool(name="singles", bufs=1))
            sb = ctx.enter_context(tc.tile_pool(name="sb", bufs=2))
            sb1 = ctx.enter_context(tc.tile_pool(name="sb1", bufs=3))
            sbig = ctx.enter_context(tc.tile_pool(name="sbig", bufs=2))
            sbig1 = ctx.enter_context(tc.tile_pool(name="sbig1", bufs=3))
            ps_z = ctx.enter_context(
                tc.tile_pool(name="ps_z", bufs=3, space="PSUM")
            )
            ps_tr = ctx.enter_context(
                tc.tile_pool(name="ps_tr", bufs=2, space="PSUM")
            )
            ps_h = ctx.enter_context(
                tc.tile_pool(name="ps_h", bufs=3, space="PSUM")
            )
            dr = ctx.enter_context(tc.tile_pool(name="dr", bufs=2, space="DRAM"))

            # ---- one-time constants ----
            identity_bf = singles.tile([128, 128], bf16)
            make_identity(nc, identity_bf)

            # One-time loads go on the Act DGE queue so the SP queue starts
            # batch 0's C/Q loads immediately. Merged DMAs: w -> [128, 6]
            # (w1|w2|w3 as column pairs), W_res -> [128, 2048].
            w_cols = singles.tile([128, 3 * KH], f32)
            nc.scalar.dma_start(
                out=w_cols,
                in_=w.ap().rearrange("(i p) -> p i", i=3 * KH, p=128),
            )
            w2_col = w_cols[:, KH : 2 * KH]
            w3_col = w_cols[:, 2 * KH : 3 * KH]
            # r-bias matmuls run in bf16 (fp32r forbids N=1); the bf16
            # rounding of r cancels in the row softmax entirely and is a
            # ~0.3% perturbation of the column softmax.
            w1_col_bf = singles.tile([128, KH], bf16)
            nc.vector.tensor_copy(w1_col_bf, w_cols[:, 0:KH])
            b_col = singles.tile([128, KH], f32)
            nc.scalar.dma_start(
                out=b_col, in_=b_res.ap().rearrange("(i p) -> p i", i=KH, p=128)
            )

            # W_res^T (bf16): WT[f][p, ho] = W_res[ho, 128*f + p]
            WT = []
            for f in range(8):
                t_wt = singles.tile([128, H], bf16, tag=f"wt{f}")
                WT.append(t_wt)
            w_nat = singles.tile([128, 2 * 4 * H], f32)
            for j in range(KH):
                nc.scalar.dma_start(
                    out=w_nat[:, 4 * H * j : 4 * H * (j + 1)],
                    in_=W_res.ap()[128 * j : 128 * (j + 1), :],
                )
            for j in range(KH):
                tb = singles.tile([128, 4 * H], bf16, tag=f"wnb{j}")
                nc.vector.tensor_copy(
                    tb, w_nat[:, 4 * H * j : 4 * H * (j + 1)]
                )
                for g in range(2):  # 4 transposes per grouped psum tile
                    pt = ps_tr.tile([128, 512], bf16, tag="tr")
                    for t4 in range(4):
                        f = 4 * g + t4
                        nc.tensor.transpose(
                            pt[:, 128 * t4 : 128 * (t4 + 1)],
                            tb[:, 128 * f : 128 * (f + 1)],
                            identity_bf,
                        )
                    for t4 in range(4):
                        f = 4 * g + t4
                        nc.vector.tensor_copy(
                            out=WT[f][:, 128 * j : 128 * (j + 1)],
                            in_=pt[:, 128 * t4 : 128 * (t4 + 1)],
                        )

            def frontend_load(b):
                """DMA loads (SP queue) + Pool elementwise: CA first, then bf16."""
                st = {}
                C_nat = []
                Q_nat = []
                for k in range(KH):
                    t = sbig.tile([128, LC], f32r, tag=f"cnat{k}")
                    nc.sync.dma_start(
                        out=t,
                        in_=C.ap()[b, 128 * k : 128 * (k + 1), :].bitcast(f32r),
                    )
                    C_nat.append(t)
                    tq = sb.tile([128, LQ], f32r, tag=f"qnat{k}")
                    nc.sync.dma_start(
                        out=tq,
                        in_=Q.ap()[b, 128 * k : 128 * (k + 1), :].bitcast(f32r),
                    )
                    Q_nat.append(tq)

                # CA = C*w3 + w2 first: it gates the Z matmuls.
                CA = []
                for k in range(KH):
                    t = sbig.tile([128, LC], f32r, tag=f"ca{k}")
                    nc.vector.tensor_scalar(
                        out=t,
                        in0=C_nat[k],
                        scalar1=w3_col[:, k : k + 1],
                        scalar2=w2_col[:, k : k + 1],
                        op0=OP.mult,
                        op1=OP.add,
                    )
                    CA.append(t)
                C_bf = []
                for k in range(KH):
                    cb = sbig.tile([128, LC], bf16, tag=f"cbf{k}")
                    nc.gpsimd.tensor_copy(cb, C_nat[k].bitcast(f32))
                    C_bf.append(cb)
                Q_bf = []
                for k in range(KH):
                    qb = sb.tile([128, LQ], bf16, tag=f"qbf{k}")
                    nc.gpsimd.tensor_copy(qb, Q_nat[k].bitcast(f32))
                    Q_bf.append(qb)

                st.update(
                    C_nat=C_nat, Q_nat=Q_nat, C_bf=C_bf, Q_bf=Q_bf, CA=CA
                )
                return st

            def frontend_r(b, st):
                """r bias in column layout: r_col[p, i] = r[128i+p]. PE+Act."""
                C_bf_r = st["C_bf"]
                ps_r = ps_z.tile([128, NLT], f32, tag="z")
                for i in range(NLT):
                    for k in range(KH):
                        mmb(
                            ps_r[:, i : i + 1],
                            C_bf_r[k][:, 128 * i : 128 * (i + 1)],
                            w1_col_bf[:, k : k + 1],
                            (k == 0),
                            (k == KH - 1),
                        )
                r_col = sb.tile([128, NLT], f32, tag="rcol")
                nc.scalar.activation(out=r_col, in_=ps_r, func=AF.Identity)
                st["r_col"] = r_col

            def frontend_tr(b, st):
                """C/Q PE transposes (grouped) -> CtT, QT."""
                C_bf = st["C_bf"]
                Q_bf = st["Q_bf"]
                CtT = sb1.tile([128, 2 * LC], bf16, tag="ctt")
                for g in range(4):
                    pt = ps_tr.tile([128, 512], bf16, tag="tr")
                    for t4 in range(4):
                        i = 2 * g + t4 // 2
                        k = t4 % 2
                        nc.tensor.transpose(
                            pt[:, 128 * t4 : 128 * (t4 + 1)],
                            C_bf[k][:, 128 * i : 128 * (i + 1)],
                            identity_bf,
                        )
                    nc.vector.tensor_copy(
                        out=CtT[:, 512 * g : 512 * (g + 1)], in_=pt
                    )
                QT = sb1.tile([128, 2 * LQ], bf16, tag="qt")
                pt = ps_tr.tile([128, 512], bf16, tag="tr")
                for t4 in range(4):
                    j = t4 // 2
                    k = t4 % 2
                    nc.tensor.transpose(
                        pt[:, 128 * t4 : 128 * (t4 + 1)],
                        Q_bf[k][:, 128 * j : 128 * (j + 1)],
                        identity_bf,
                    )
                nc.scalar.activation(out=QT, in_=pt, func=AF.Identity)
                st["CtT"] = CtT
                st["QT"] = QT

            def backend(b, st, emit_next_r=None, emit_next_tr=None):
                C_nat = st["C_nat"]; Q_nat = st["Q_nat"]; C_bf = st["C_bf"]
                CtT = st["CtT"]; QT = st["QT"]; CA = st["CA"]
                r_col = st["r_col"]

                rho_col = sb.tile([128, NLT], f32, tag="rho")
                rho_inv = sb.tile([128, NLT], f32, tag="rhoi")
                rho_inv_bf = sb.tile([128, NLT], bf16, tag="rhoib")
                ri_dram = dr.tile([1, LC], bf16, tag="rid")
                ri_bc = sbig1.tile([128, LC], bf16, tag="ribc")
                ri_rearr = ri_dram.rearrange("1 (i p) -> p i", i=NLT, p=128)
                kap_parts = sb.tile([128, 4], f32, tag="kapp")
                E_lm = []
                E_ml = []
                for j in range(NMT):
                    e = sbig1.tile([128, LC], bf16, tag=f"eml{j}")
                    E_ml.append(e)

                def z_tile(i):
                    pz = ps_z.tile([128, LQ], f32, tag="z")
                    for k in range(KH):
                        mm(
                            pz,
                            CA[k][:, 128 * i : 128 * (i + 1)],
                            Q_nat[k],
                            (k == 0),
                            (k == KH - 1),
                        )
                    e = sb1.tile([128, LQ], bf16, tag=f"elm{i}")
                    nc.scalar.activation(
                        out=e,
                        in_=pz,
                        func=AF.Exp,
                        bias=r_col[:, i : i + 1],
                        accum_out=rho_col[:, i : i + 1],
                    )
                    E_lm.append(e)

                def e_transpose(half, j, copy_eng):
                    # E_ml[j][:, 512h:512h+512] = transpose of E_lm[4h..4h+3] j-block
                    pt = ps_tr.tile([128, 512], bf16, tag="tr")
                    for t4 in range(4):
                        i = 4 * half + t4
                        nc.tensor.transpose(
                            pt[:, 128 * t4 : 128 * (t4 + 1)],
                            E_lm[i][:, 128 * j : 128 * (j + 1)],
                            identity_bf,
                        )
                    # Act copy w/ accum: kappa partial colsum over this l-range
                    nc.scalar.activation(
                        out=E_ml[j][:, 512 * half : 512 * (half + 1)],
                        in_=pt,
                        func=AF.Identity,
                        accum_out=kap_parts[:, 2 * j + half : 2 * j + half + 1],
                    )

                def rho_half(h):
                    # reciprocal + bf16 + DRAM bounce for l-range [512h, 512h+512)
                    hs = slice(4 * h, 4 * (h + 1))
                    nc.vector.reciprocal(rho_inv[:, hs], rho_col[:, hs])
                    nc.vector.tensor_copy(rho_inv_bf[:, hs], rho_inv[:, hs])
                    nc.scalar.dma_start(
                        out=ri_rearr[:, hs], in_=rho_inv_bf[:, hs]
                    )
                    bc_src = bass.AP(
                        tensor=ri_dram.tensor,
                        offset=ri_dram.offset + 512 * h,
                        ap=[[0, 128], [1, 512]],
                    )
                    nc.scalar.dma_start(
                        out=ri_bc[:, 512 * h : 512 * (h + 1)], in_=bc_src
                    )

                # ---- Z tiles with E-transposes and rho bounces woven in ----
                for i in range(6):
                    z_tile(i)
                rho_half(0)
                e_transpose(0, 0, "dve")
                z_tile(6)
                e_transpose(0, 1, "dve")
                z_tile(7)
                rho_half(1)
                if emit_next_r is not None:
                    emit_next_r()
                e_transpose(1, 0, "act")
                e_transpose(1, 1, "act")

                kap_col = sb.tile([128, NMT], f32, tag="kap")
                for j in range(NMT):
                    nc.vector.tensor_add(
                        kap_col[:, j : j + 1],
                        kap_parts[:, 2 * j : 2 * j + 1],
                        kap_parts[:, 2 * j + 1 : 2 * j + 2],
                    )
                kap_inv = sb.tile([128, NMT], f32, tag="kapi")
                nc.vector.reciprocal(kap_inv, kap_col)

                # ---- T = S_col^T @ Ct   [m, h] ----
                T_nat = []
                for j in range(NMT):
                    pT = ps_z.tile([128, H], f32, tag="z")
                    for i in range(NLT):
                        mmb(
                            pT,
                            E_lm[i][:, 128 * j : 128 * (j + 1)],
                            CtT[:, 256 * i : 256 * (i + 1)],
                            (i == 0),
                            (i == NLT - 1),
                        )
                    t = sb1.tile([128, H], bf16, tag=f"tn{j}")
                    nc.vector.tensor_scalar_mul(t, pT, kap_inv[:, j : j + 1])
                    T_nat.append(t)

                # ---- A^T [h, l] with 1/rho fold + Ct*A ----
                A_T = []
                Bv_T = []
                CA1 = []
                CB1 = []
                for t_i in range(KH):
                    t_a = sbig1.tile([128, LC], bf16, tag=f"at{t_i}")
                    t_ca1 = sbig1.tile([128, LC], bf16, tag=f"ca1{t_i}")
                    t_bv = sbig1.tile([128, LC], bf16, tag=f"bvt{t_i}")
                    t_cb1 = sbig1.tile([128, LC], bf16, tag=f"cb1{t_i}")
                    A_T.append(t_a)
                    CA1.append(t_ca1)
                    Bv_T.append(t_bv)
                    CB1.append(t_cb1)
                for c in range(2):
                    cs = slice(512 * c, 512 * (c + 1))
                    for t_i in range(KH):
                        pA = ps_h.tile([128, 512], f32, tag="h")
                        for k in range(NMT):
                            mmb(
                                pA,
                                QT[:, 256 * k + 128 * t_i : 256 * k + 128 * (t_i + 1)],
                                E_ml[k][:, cs],
                                (k == 0),
                                (k == NMT - 1),
                            )
                        nc.vector.tensor_mul(A_T[t_i][:, cs], pA, ri_bc[:, cs])
                        nc.vector.tensor_mul(
                            CA1[t_i][:, cs], C_bf[t_i][:, cs], A_T[t_i][:, cs]
                        )

                # ---- Bv^T [h, l] with 1/rho fold + Ct*Bv ----
                for c in range(2):
                    cs = slice(512 * c, 512 * (c + 1))
                    for t_i in range(KH):
                        pB = ps_h.tile([128, 512], f32, tag="h")
                        for k in range(NMT):
                            mmb(
                                pB,
                                T_nat[k][:, 128 * t_i : 128 * (t_i + 1)],
                                E_ml[k][:, cs],
                                (k == 0),
                                (k == NMT - 1),
                            )
                        nc.vector.tensor_mul(Bv_T[t_i][:, cs], pB, ri_bc[:, cs])
                        nc.vector.tensor_mul(
                            CB1[t_i][:, cs], C_bf[t_i][:, cs], Bv_T[t_i][:, cs]
                        )

                # next batch's C/Q transposes fill the PE while DVE drains
                if emit_next_tr is not None:
                    emit_next_tr()

                # ---- final matmul + relu + store ----
                blocks = [
                    C_bf[0],
                    C_bf[1],
                    A_T[0],
                    A_T[1],
                    CA1[0],
                    CA1[1],
                    CB1[0],
                    CB1[1],
                ]
                o_t = []
                for t_i in range(KH):
                    o = sbig.tile([128, LC], f32, tag=f"osb{t_i}")
                    o_t.append(o)
                for c in range(2):
                    cs = slice(512 * c, 512 * (c + 1))
                    for t_i in range(KH):
                        po = ps_h.tile([128, 512], f32, tag="h")
                        for f in range(8):
                            mmb(
                                po,
                                WT[f][:, 128 * t_i : 128 * (t_i + 1)],
                                blocks[f][:, cs],
                                (f == 0),
                                (f == 7),
                            )
                        nc.scalar.activation(
                            out=o_t[t_i][:, cs],
                            in_=po,
                            func=AF.Relu,
                            bias=b_col[:, t_i : t_i + 1],
                        )
                        if c == 1:
                            nc.scalar.dma_start(
                                out=out.ap()[b, 128 * t_i : 128 * (t_i + 1), :],
                                in_=o_t[t_i],
                            )

            def body(iv=None):
                sts = {}
                sts[0] = frontend_load(0)
                frontend_r(0, sts[0])
                frontend_tr(0, sts[0])
                for b in range(BB):
                    if b + 1 < BB:
                        sts[b + 1] = frontend_load(b + 1)
                        nr = lambda bn=b + 1: frontend_r(bn, sts[bn])
                        nt = lambda bn=b + 1: frontend_tr(bn, sts[bn])
                    else:
                        nr = nt = None
                    backend(b, sts[b], emit_next_r=nr, emit_next_tr=nt)
                    del sts[b]

            if reps == 1:
                body()
            else:
                with tc.For_i(0, reps, 1) as iv:
                    body(iv)

    nc.compile()
    return nc


def _get_nc(reps: int = 1):
    key = ("nc", reps)
    if key not in _CACHE:
        _CACHE[key] = _build(reps)
    return _CACHE[key]


def kernel(C, Q, cmask, qmask, w, W_res, b_res, _reps: int = 1, _want_res: bool = False):
    from concourse.bass_utils import run_bass_kernel_spmd

    nc = _get_nc(_reps)

    C = np.ascontiguousarray(C, dtype=np.float32)
    Q = np.ascontiguousarray(Q, dtype=np.float32)
    w = np.ascontiguousarray(w, dtype=np.float32)
    W_res = np.ascontiguousarray(W_res, dtype=np.float32)
    b_res = np.ascontiguousarray(b_res, dtype=np.float32)

    in_maps = []
    for i in range(N_CORES):
        sl = slice(i * BB, (i + 1) * BB)
        in_maps.append(
            {"C": C[sl], "Q": Q[sl], "w": w, "W_res": W_res, "b_res": b_res}
        )

    res = run_bass_kernel_spmd(nc, in_maps, core_ids=list(range(N_CORES)))
    out = np.concatenate([res.results[i]["out"] for i in range(N_CORES)], axis=0)
    if _want_res:
        return out, res
    return out

